# revision 7
# baseline (speedup 1.0000x reference)
"""MaxIoUAssigner Trainium2 kernel (8 NeuronCores, SPMD over anchors).

Contract: kernel(**inputs) takes the FULL inputs
  bboxes  [500000, 4] f32
  targets [128, 5]    f32   (x1,y1,x2,y2,label; label==-1 => invalid GT)
  num_level_bboxes    (unused by the reference computation)
and returns the FULL outputs (assigned int32 [N], max_overlaps f32 [N],
assigned_labels int32 [N]) matching the jax reference.

Design v5 ("fp16 streams, DVE+ScalarE split"):
  Anchors are y-sorted and laid out [128 partitions x C columns] per core
  (rank r -> col r//1024, core r%8, part (r%1024)//8). For each valid GT j
  (sorted by gy1) only a contiguous column slice [lo, hi) can overlap it
  (even-aligned so fp16 ops hit the DVE 2x_1P packed mode). Per GT the
  device runs a 7-op chain over its slice in scaled-w space
  (w' = 1024*q/(1+q) = 1024*inter/(area_b+area_g), monotone in IoU q;
  the 1024 scale keeps 1/(ab+ag) inside fp16 normal range):
    ScalarE: rs  = Recip(ab/1024 + ag/1024)   f32 in -> fp16 out (~2.4e-4)
    DVE:     xd  = EXTENT(bx2,bx1 | gx2,gx1)  custom, f32 in -> fp16 out
             yd  = EXTENT(by2,by1 | gy2,gy1)
             it  = xd*yd                      builtin TT mult, fp16 2x
             qv  = it*rs                      builtin TT mult, fp16 2x
             colmax_j = reduce_max(qv)        builtin reduce, fp16
             acc = max(acc, qv)               builtin TT max, fp16 2x
  Outputs: maxw' [P,C+1] fp16 and colmax [P,G] fp16. Host: combine, w->q,
  pos/neg thresholds with a +-2.5e-3 exact-recompute window, exact f32 row
  argmax for the ~2.7% positive anchors, and the reference per-GT overwrite
  pass where every (core,partition) whose device colmax is within 4e-3 rel
  of the global max gets an exact f32 recompute (device error ~1e-3 can
  exceed the data's top-2 margins, the slack window restores exactness).
"""

import sys

import numpy as np

sys.path.insert(0, "/opt/trn_rl_repo")

import concourse.bass as bass
import concourse.bacc as bacc
import concourse.mybir as mybir
from concourse import dve_ops
from concourse import tile
from concourse.bass_utils import run_bass_kernel_spmd
from concourse.dve_spec import Spec, Src0, Src1, Zero, lower, maxx, minn, relu
from concourse.dve_spec import C0 as DC0
from concourse.dve_spec import C1 as DC1
from concourse.dve_spec import _has_src1
from concourse.dve_uop import DveOpSpec
from concourse.dve_ops import DveOp

# ----------------------------------------------------------------------------
# Problem constants (hardcoded per the harness contract)
# ----------------------------------------------------------------------------
N_FULL = 500000
G = 128
N_CORES = 8
P = 128  # SBUF partitions
C = 489  # real anchor columns per partition per core
CD = 490  # device columns (even; col 489 = degenerate pad)
N_CORE = P * C  # 62592 anchors per core (padded)
N_PAD = N_CORE * N_CORES  # 500736
POS_THR = 0.5
NEG_THR = 0.4
THR_TOL = 2.5e-3  # flag |q - thr| < tol for exact host recompute
COL_SLACK = 4e-3  # overwrite-pass candidate window (rel)
WSCALE = 1024.0

F32 = mybir.dt.float32
F16 = mybir.dt.float16
AF = mybir.AluOpType
ACT = mybir.ActivationFunctionType


# ----------------------------------------------------------------------------
# Custom fused DVE ops (registered at import)
# ----------------------------------------------------------------------------
def _register_custom_op(name: str, spec: Spec, subdim: bool = False) -> DveOp:
    existing = {op.name: op for op in dve_ops.OPS}
    if name in existing:
        return existing[name]
    row = max(dve_ops._SUB_OPCODE_FOR_NAME.values()) + 1
    assert row < 0x20, "custom-DVE opcode rows exhausted"
    dve_ops._SUB_OPCODE_FOR_NAME[name] = row
    op = DveOp(name, spec, subdim=subdim, uops_sha={})
    for ver in ("v3", "v4"):
        tmp = DveOpSpec(
            name=name, opcode=row, uops=lower(spec, ver=ver), rd1_en=_has_src1(spec)
        )
        op.uops_sha[ver] = tmp.sha(ver)
    dve_ops.OPS.append(op)
    dve_ops.CUSTOM_DVE_SPECS[name] = spec
    return op


# clipped extent: relu(min(Src0, s0) - max(Src1, s1))
EXTENT = _register_custom_op(
    "IOU_EXTENT",
    Spec(
        body=relu(minn(Src0, DC0) - maxx(Src1, DC1)),
        reference=lambda in0, in1, c0, c1, c2: np.maximum(
            np.float32(np.minimum(in0, c0) - np.maximum(in1, c1)), np.float32(0)
        ),
    ),
)


def _scalar_act_raw(nc, out, in_, func, bias=0.0, scale=1.0, alpha=0.0):
    """Emit InstActivation directly (the bass wrapper forbids Reciprocal)."""
    eng = nc.scalar
    ins = [eng.lower_ap(in_)]
    for arg in (bias, scale, alpha):
        ins.append(mybir.ImmediateValue(dtype=mybir.dt.float32, value=float(arg)))
    return eng.add_instruction(
        mybir.InstActivation(
            name=nc.get_next_instruction_name(),
            func=func,
            ins=ins,
            outs=[eng.lower_ap(out)],
        )
    )


# ----------------------------------------------------------------------------
# Device program
# ----------------------------------------------------------------------------
def build_program(
    cols: int,
    slices: tuple,  # per sorted-GT (lo, hi) even-aligned; (0, 0) = skip
    gvals: tuple,  # per sorted-GT (gx1, gy1, gx2, gy2, area_g) f32
) -> bass.Bass:
    """Per-core SPMD Bass program (identical on all cores; per-core data).

    bb  [5, P, cols]: x1, y1, x2, y2, area_b   (f32)
    out_maxw  [P, cols] fp16: row max in scaled-w space
    out_small [P, G]    fp16: per-GT core-local column max (scaled-w)
    """
    nc = bacc.Bacc(
        "TRN2", target_bir_lowering=False, debug=False, num_devices=N_CORES
    )

    bb = nc.declare_dram_parameter("bb", [5, P, cols], F32, isOutput=False)
    out_maxw = nc.declare_dram_parameter("out_maxw", [P, cols], F16, isOutput=True)
    out_small = nc.declare_dram_parameter("out_small", [P, G], F16, isOutput=True)

    BX1, BY1, BX2, BY2, AREAB = range(5)

    lmax = max([hi - lo for (lo, hi) in slices] + [2])
    n_acc = 4  # independent running-max accumulators

    with tile.TileContext(nc) as tc:
        with (
            tc.tile_pool(name="const", bufs=1) as constp,
            tc.tile_pool(name="rsp", bufs=8) as rsp,
            tc.tile_pool(name="work", bufs=6) as work,
        ):
            # ---- constants / inputs -------------------------------------
            # chunked plane DMAs, extent planes first, so the first GT
            # chains start as soon as their columns have landed
            bbt = [
                constp.tile([P, cols], F32, tag=f"bb{k}", name=f"bb{k}")
                for k in range(5)
            ]
            half = (cols // 2) & ~1
            for k in (2, 0, 3, 1, 4):  # x2, x1, y2, y1, area_b
                nc.sync.dma_start(bbt[k][:, :half], bb[k][:, :half])
            for k in (2, 0, 3, 1, 4):
                nc.sync.dma_start(bbt[k][:, half:], bb[k][:, half:])

            colmax = constp.tile([P, G], F16, tag="colmax", name="colmax")
            nc.scalar.memzero(colmax[:])
            maxq4 = [
                constp.tile([P, cols], F16, tag=f"maxq{k}", name=f"maxq{k}")
                for k in range(n_acc)
            ]
            for k in range(n_acc):
                nc.scalar.memzero(maxq4[k][:])

            # ---- per-GT chains ------------------------------------------
            for jj, (lo, hi) in enumerate(slices):
                if hi <= lo:
                    continue
                L = hi - lo
                S = slice(lo, hi)
                gx1, gy1, gx2, gy2, areag = gvals[jj]
                rs = rsp.tile([P, lmax], F16, tag="rs", name=f"rs{jj}")
                _scalar_act_raw(
                    nc, rs[:, :L], bbt[AREAB][:, S], ACT.Reciprocal,
                    bias=areag / WSCALE, scale=1.0 / WSCALE,
                )
                xd = work.tile([P, lmax], F16, tag="xd", name=f"xd{jj}")
                yd = work.tile([P, lmax], F16, tag="yd", name=f"yd{jj}")
                it = work.tile([P, lmax], F16, tag="it", name=f"it{jj}")
                qv = work.tile([P, lmax], F16, tag="qv", name=f"qv{jj}")
                nc.vector._custom_dve(
                    EXTENT, out=xd[:, :L], in0=bbt[BX2][:, S],
                    in1=bbt[BX1][:, S], s0=gx2, s1=gx1,
                )
                nc.vector._custom_dve(
                    EXTENT, out=yd[:, :L], in0=bbt[BY2][:, S],
                    in1=bbt[BY1][:, S], s0=gy2, s1=gy1,
                )
                nc.vector.tensor_tensor(
                    out=it[:, :L], in0=xd[:, :L], in1=yd[:, :L], op=AF.mult
                )
                nc.vector.tensor_tensor(
                    out=qv[:, :L], in0=it[:, :L], in1=rs[:, :L], op=AF.mult
                )
                nc.vector.tensor_reduce(
                    out=colmax[:, jj : jj + 1], in_=qv[:, :L],
                    axis=mybir.AxisListType.X, op=AF.max,
                )
                mk = maxq4[jj % n_acc]
                nc.vector.tensor_tensor(
                    out=mk[:, S], in0=mk[:, S], in1=qv[:, :L], op=AF.max
                )

            # ---- fold accumulators, write outputs -----------------------
            maxw = constp.tile([P, cols], F16, tag="maxw", name="maxw")
            st = 1
            while st < n_acc:
                for a in range(0, n_acc, 2 * st):
                    dst = maxq4[a][:] if 2 * st < n_acc else maxw[:]
                    nc.vector.tensor_tensor(
                        out=dst, in0=maxq4[a][:], in1=maxq4[a + st][:], op=AF.max
                    )
                st *= 2
            nc.sync.dma_start(out_maxw[0:P], maxw[:])
            nc.sync.dma_start(out_small[0:P], colmax[:])

    nc.compile()
    return nc


# ----------------------------------------------------------------------------
# Host-side input prep / output gather / fixup
# ----------------------------------------------------------------------------
_NC_CACHE: dict = {}
LAST_RESULTS = None


def _iou_rows(bb_rows: np.ndarray, targets: np.ndarray, valid: np.ndarray):
    """Exact f32 replica of the reference IoU for a subset of anchors."""
    f32 = np.float32
    fx1, fy1 = bb_rows[:, 0:1], bb_rows[:, 1:2]
    fx2, fy2 = bb_rows[:, 2:3], bb_rows[:, 3:4]
    tgx1, tgy1 = targets[None, :, 0], targets[None, :, 1]
    tgx2, tgy2 = targets[None, :, 2], targets[None, :, 3]
    iw = np.maximum(np.minimum(fx2, tgx2) - np.maximum(fx1, tgx1), f32(0)).astype(f32)
    ih = np.maximum(np.minimum(fy2, tgy2) - np.maximum(fy1, tgy1), f32(0)).astype(f32)
    fint = (iw * ih).astype(f32)
    fab = ((fx2 - fx1) * (fy2 - fy1)).astype(f32)
    fag = ((tgx2 - tgx1) * (tgy2 - tgy1)).astype(f32)
    fov = (fint / (fab + fag - fint + f32(1e-16))).astype(f32)
    return np.where(valid[None, :], fov, f32(-1.0))


def kernel(bboxes: np.ndarray, targets: np.ndarray, num_level_bboxes=None):
    f32 = np.float32
    bboxes = np.asarray(bboxes, dtype=f32)
    targets = np.asarray(targets, dtype=f32)
    n = bboxes.shape[0]
    assert n == N_FULL, f"kernel hardcoded for N={N_FULL}, got {n}"

    # Pad with degenerate far-away anchors (IoU 0 with every GT).
    pad = np.full((N_PAD - n, 4), 2000.0, dtype=f32)
    bb_all = np.concatenate([bboxes, pad], axis=0)  # [N_PAD, 4]

    # y-sort anchors; rank r -> (col r//1024, core r%8, part (r%1024)//8)
    perm = np.argsort(bb_all[:, 1], kind="stable")
    bbs = bb_all[perm]
    ys = bbs[:, 1]
    maxhb = float((bboxes[:, 3] - bboxes[:, 1]).max()) + 1e-3

    # GT slot order: valid GTs sorted by gy1 (invalid get empty slices)
    lab = targets[:, 4]
    valid = lab != f32(-1.0)
    gy1key = np.where(valid, targets[:, 1], f32(1e9))
    gorder = np.argsort(gy1key, kind="stable")

    slices = []
    for j in gorder:
        if not valid[j]:
            slices.append((0, 0))
            continue
        gy1, gy2 = float(targets[j, 1]), float(targets[j, 3])
        lo = int(np.searchsorted(ys, gy1 - maxhb, "left")) // 1024
        hi = (int(np.searchsorted(ys, gy2, "right")) + 1023) // 1024
        hi = max(min(hi, C), 1)
        lo = max(0, min(lo, hi - 1))
        lo &= ~1  # even alignment for fp16 2x mode
        if (hi - lo) % 2:
            hi += 1  # <= CD
        slices.append((lo, hi))
    slices = tuple(slices)

    # ---- device inputs ------------------------------------------------
    # bb [cores][5, P, CD]: x1, y1, x2, y2, area_b (+1 degenerate pad col)
    arr = bbs.reshape(C, P, N_CORES, 4)  # [c, p, m, k]
    area_b = (
        (arr[..., 2] - arr[..., 0]) * (arr[..., 3] - arr[..., 1])
    ).astype(f32)  # [c, p, m]
    shards = []
    for m in range(N_CORES):
        sh = np.full((5, P, CD), 2000.0, dtype=f32)
        for k in range(4):
            sh[k, :, :C] = arr[:, :, m, k].T
        sh[4, :, :C] = area_b[:, :, m].T
        sh[4, :, C] = 0.0
        shards.append(sh)

    # GT scalars (slot = sorted order), baked into the program as imms.
    t = targets
    gx1 = t[gorder, 0].astype(f32)
    gy1 = t[gorder, 1].astype(f32)
    gx2 = t[gorder, 2].astype(f32)
    gy2 = t[gorder, 3].astype(f32)
    area_g = ((gx2 - gx1) * (gy2 - gy1)).astype(f32)
    gvals = tuple(
        (float(gx1[s]), float(gy1[s]), float(gx2[s]), float(gy2[s]), float(area_g[s]))
        for s in range(G)
    )

    key = (CD, slices, gvals)
    if key not in _NC_CACHE:
        _NC_CACHE.clear()
        _NC_CACHE[key] = build_program(CD, slices, gvals)
    nc = _NC_CACHE[key]
    in_maps = [{"bb": shards[m]} for m in range(N_CORES)]
    res = run_bass_kernel_spmd(nc, in_maps, core_ids=list(range(N_CORES)))
    global LAST_RESULTS
    LAST_RESULTS = res

    maxw_dev = np.stack([r["out_maxw"] for r in res.results])  # [m, P, CD] fp16
    small = np.stack([r["out_small"] for r in res.results])  # [m, P, G] fp16

    # unshard maxw (scaled w): sorted rank r = c*1024 + p*8 + m
    sorted_w = maxw_dev[:, :, :C].transpose(2, 1, 0).reshape(N_PAD)
    w_full = np.empty(N_PAD, np.float64)
    w_full[perm] = sorted_w.astype(np.float64)
    w = w_full[:n] / WSCALE

    # w -> q conversion (w = q/(1+q)); device w has ~1e-3 rel error
    max_ov = (w / (1.0 - w)).astype(f32)

    # ---- host: thresholds with exact recompute near the boundaries ----
    flag = np.nonzero(
        (np.abs(max_ov - POS_THR) < THR_TOL) | (np.abs(max_ov - NEG_THR) < THR_TOL)
    )[0]
    if len(flag):
        fov = _iou_rows(bboxes[flag], targets, valid)
        max_ov[flag] = fov.max(1)

    pos_mask = max_ov > f32(POS_THR)
    neg_mask = max_ov < f32(NEG_THR)

    assigned = np.full(n, -1, dtype=np.int32)
    assigned[neg_mask] = 0

    # ---- host: exact argmax rows for the positive anchors -------------
    pos_idx = np.nonzero(pos_mask)[0]
    if len(pos_idx):
        fov = _iou_rows(bboxes[pos_idx], targets, valid)
        max_ov[pos_idx] = fov.max(1)
        assigned[pos_idx] = fov.argmax(1).astype(np.int32) + 1

    # ---- host: the reference's per-GT overwrite pass -------------------
    # Device colmax error (~1e-3) can exceed the data's top-2 margins, so
    # every (core, partition) within COL_SLACK of the global max gets an
    # exact f32 recompute; the true max among the candidates wins.
    slot_of_j = np.empty(G, dtype=int)
    slot_of_j[gorder] = np.arange(G)
    arrv = bbs.reshape(C, P, N_CORES, 4)  # sorted-layout anchor coords
    for j in range(G):
        if not valid[j]:
            continue
        s = slot_of_j[j]
        col = small[:, :, s].astype(np.float64)  # [m, P]
        glob = float(col.max())
        if glob <= 0.0:
            continue
        gx1j, gy1j, gx2j, gy2j = (float(targets[j, k]) for k in range(4))
        agj = np.float32(
            (np.float32(gx2j) - np.float32(gx1j))
            * (np.float32(gy2j) - np.float32(gy1j))
        )
        lo, hi = slices[s]
        hi = min(hi, C)
        best_q = -1.0
        best_r = -1
        for m, p in zip(*np.nonzero(col >= glob * (1.0 - COL_SLACK))):
            row = arrv[lo:hi, p, m, :]  # [L, 4] f32
            iw = np.minimum(row[:, 2], np.float32(gx2j)) - np.maximum(
                row[:, 0], np.float32(gx1j)
            )
            ih = np.minimum(row[:, 3], np.float32(gy2j)) - np.maximum(
                row[:, 1], np.float32(gy1j)
            )
            iw = np.maximum(iw, np.float32(0)).astype(np.float32)
            ih = np.maximum(ih, np.float32(0)).astype(np.float32)
            inter_r = (iw * ih).astype(np.float32)
            ab = ((row[:, 2] - row[:, 0]) * (row[:, 3] - row[:, 1])).astype(
                np.float32
            )
            q = (inter_r / (ab + agj - inter_r + np.float32(1e-16))).astype(
                np.float32
            )
            c = int(np.argmax(q))
            if float(q[c]) > best_q:
                best_q = float(q[c])
                best_r = (lo + c) * 1024 + int(p) * 8 + int(m)
        if best_r >= 0:
            a = int(perm[best_r])
            if a < n:
                assigned[a] = j + 1

    labels = np.where(
        assigned > 0,
        lab[np.clip(assigned - 1, 0, G - 1)].astype(np.int32),
        -1,
    ).astype(np.int32)
    return assigned, max_ov, labels


if __name__ == "__main__":
    inp = {
        "bboxes": np.load("/root/problem/ref_bboxes.npy"),
        "targets": np.load("/root/problem/ref_targets.npy"),
        "num_level_bboxes": 5,
    }
    a, m, l = kernel(**inp)
    print("assigned", a[:10], "maxov", m[:5], "labels", l[:10])


# revision 8
# speedup vs baseline: 1.1331x; 1.1331x over previous
"""MaxIoUAssigner Trainium2 kernel (8 NeuronCores, SPMD over anchors).

Contract: kernel(**inputs) takes the FULL inputs
  bboxes  [500000, 4] f32
  targets [128, 5]    f32   (x1,y1,x2,y2,label; label==-1 => invalid GT)
  num_level_bboxes    (unused by the reference computation)
and returns the FULL outputs (assigned int32 [N], max_overlaps f32 [N],
assigned_labels int32 [N]) exactly like the jax reference.

Design v2 ("lean slab", DVE+ScalarE split):
  Anchors are y-sorted and laid out [128 partitions x C columns] per core
  (rank r -> col r//1024, core r%8, part (r%1024)//8). For each valid GT j
  (sorted by gy1) only a contiguous column slice [lo, hi) can overlap it.
  Per GT the device runs a 6-op chain over its slice in w-space
  (w = q/(1+q) = inter/(area_b+area_g), strictly monotone in IoU q):
    ScalarE: rs  = Reciprocal(area_b + area_g)   (fused act bias; ~1.2e-5 rel)
    DVE:     xd  = EXTENT(bx2,bx1 | gx2,gx1)     relu'd x-extent
             yd  = EXTENT(by2,by1 | gy2,gy1)
             it  = RELUMUL(xd, yd)               intersection
             q   = MUL_MAXRED(it, rs)            w values; accum -> colmax[P,1]
             acc = MAX2(acc, q)                  running row max (4 accums)
  The only outputs are maxw [P,C] (row max in w-space) and colmax [P,G].
  Everything else moved to the host: w->q conversion, pos/neg thresholds
  (threshold-window anchors recomputed exactly in f32), the row argmax for
  the ~2.7% positive anchors (exact f32 rows, reference tie semantics), and
  the reference's per-GT overwrite pass (device colmax selects the winning
  (core, partition); the winning column is recomputed exactly; top-2 colmax
  margins on this data are ~12x the device error).
"""

import sys

import numpy as np

sys.path.insert(0, "/opt/trn_rl_repo")

import concourse.bass as bass
import concourse.bacc as bacc
import concourse.mybir as mybir
from concourse import dve_ops
from concourse import tile
from concourse.bass_utils import run_bass_kernel_spmd
from concourse.dve_spec import Spec, Src0, Src1, Zero, lower, maxx, minn, relu
from concourse.dve_spec import C0 as DC0
from concourse.dve_spec import C1 as DC1
from concourse.dve_spec import _has_src1
from concourse.dve_uop import DveOpSpec
from concourse.dve_ops import DveOp

# ----------------------------------------------------------------------------
# Problem constants (hardcoded per the harness contract)
# ----------------------------------------------------------------------------
N_FULL = 500000
G = 128
N_CORES = 8
P = 128  # SBUF partitions
C = 489  # anchor columns per partition per core
N_CORE = P * C  # 62592 anchors per core (padded)
N_PAD = N_CORE * N_CORES  # 500736
POS_THR = 0.5
NEG_THR = 0.4
THR_TOL = 1e-4  # flag |q - thr| < tol for exact host recompute

F32 = mybir.dt.float32
AF = mybir.AluOpType
ACT = mybir.ActivationFunctionType


# ----------------------------------------------------------------------------
# Custom fused DVE ops (registered at import)
# ----------------------------------------------------------------------------
def _register_custom_op(name: str, spec: Spec, subdim: bool = False) -> DveOp:
    existing = {op.name: op for op in dve_ops.OPS}
    if name in existing:
        return existing[name]
    row = max(dve_ops._SUB_OPCODE_FOR_NAME.values()) + 1
    assert row < 0x20, "custom-DVE opcode rows exhausted"
    dve_ops._SUB_OPCODE_FOR_NAME[name] = row
    op = DveOp(name, spec, subdim=subdim, uops_sha={})
    for ver in ("v3", "v4"):
        tmp = DveOpSpec(
            name=name, opcode=row, uops=lower(spec, ver=ver), rd1_en=_has_src1(spec)
        )
        op.uops_sha[ver] = tmp.sha(ver)
    dve_ops.OPS.append(op)
    dve_ops.CUSTOM_DVE_SPECS[name] = spec
    return op


# clipped extent: relu(min(Src0, s0) - max(Src1, s1))
EXTENT = _register_custom_op(
    "IOU_EXTENT",
    Spec(
        body=relu(minn(Src0, DC0) - maxx(Src1, DC1)),
        reference=lambda in0, in1, c0, c1, c2: np.maximum(
            np.float32(np.minimum(in0, c0) - np.maximum(in1, c1)), np.float32(0)
        ),
    ),
)

# inter = relu(dx) * relu(dy)  (relu is a no-op here; extents already >=0)
RELUMUL = _register_custom_op(
    "IOU_RELUMUL",
    Spec(
        body=relu(Src0) * relu(Src1),
        reference=lambda in0, in1, c0, c1, c2: np.float32(
            np.maximum(in0, np.float32(0)) * np.maximum(in1, np.float32(0))
        ),
    ),
)

# elementwise max (row-max folding)
MAX2 = _register_custom_op(
    "IOU_MAX2",
    Spec(
        body=maxx(Src0, Src1),
        reference=lambda in0, in1, c0, c1, c2: np.maximum(in0, in1),
    ),
)

# out = Src0*Src1 ; accum_out = max(out) over the free dim (init 0)
MUL_MAXRED = _register_custom_op(
    "IOU_MUL_MAXRED",
    Spec(
        body=Src0 * Src1,
        accum=maxx,
        accum_init=Zero,
        reference=lambda in0, in1, c0, c1, c2: (
            r := np.float32(in0 * in1),
            np.max(r, axis=-1, keepdims=True),
        ),
    ),
)


def _scalar_act_raw(nc, out, in_, func, bias=0.0, scale=1.0, alpha=0.0):
    """Emit InstActivation directly (the bass wrapper forbids Reciprocal)."""
    eng = nc.scalar
    ins = [eng.lower_ap(in_)]
    for arg in (bias, scale, alpha):
        ins.append(mybir.ImmediateValue(dtype=mybir.dt.float32, value=float(arg)))
    return eng.add_instruction(
        mybir.InstActivation(
            name=nc.get_next_instruction_name(),
            func=func,
            ins=ins,
            outs=[eng.lower_ap(out)],
        )
    )


# ----------------------------------------------------------------------------
# Device program
# ----------------------------------------------------------------------------
def build_program(
    cols: int,
    slices: tuple,  # per sorted-GT (lo, hi); (0, 0) = invalid GT, skipped
    gvals: tuple,  # per sorted-GT (gx1, gy1, gx2, gy2, area_g) f32
) -> bass.Bass:
    """Per-core SPMD Bass program (identical on all cores; per-core data).

    bb  [5, P, cols]: x1, y1, x2, y2, area_b
    out_maxw  [P, cols]: row max in w-space
    out_small [P, G]:    per-GT core-local column max (w-space)
    """
    nc = bacc.Bacc(
        "TRN2", target_bir_lowering=False, debug=False, num_devices=N_CORES
    )

    bb = nc.declare_dram_parameter("bb", [5, P, cols], F32, isOutput=False)
    out_maxw = nc.declare_dram_parameter("out_maxw", [P, cols], F32, isOutput=True)
    out_small = nc.declare_dram_parameter("out_small", [P, G], F32, isOutput=True)

    BX1, BY1, BX2, BY2, AREAB = range(5)

    lmax = max([hi - lo for (lo, hi) in slices] + [1])
    n_acc = 8  # independent running-max accumulators

    with tile.TileContext(nc) as tc:
        with (
            tc.tile_pool(name="const", bufs=1) as constp,
            tc.tile_pool(name="rsp", bufs=12) as rsp,
            tc.tile_pool(name="work", bufs=6) as work,
        ):
            # ---- constants / inputs -------------------------------------
            # chunked plane DMAs, extent planes first, so the first GT
            # chains start as soon as their columns have landed
            bbt = [
                constp.tile([P, cols], F32, tag=f"bb{k}", name=f"bb{k}")
                for k in range(5)
            ]
            t1, t2 = cols // 3, 2 * cols // 3
            for k in (2, 0, 3, 1, 4):  # x2, x1, y2, y1, area_b
                nc.sync.dma_start(bbt[k][:, :t1], bb[k][:, :t1])
            for k in (2, 0, 3, 1, 4):
                nc.sync.dma_start(bbt[k][:, t1:t2], bb[k][:, t1:t2])
            for k in (2, 0, 3, 1, 4):
                nc.sync.dma_start(bbt[k][:, t2:], bb[k][:, t2:])

            colmax = constp.tile([P, G], F32, tag="colmax", name="colmax")
            nc.gpsimd.memset(colmax[:], 0.0)
            maxq4 = [
                constp.tile([P, cols], F32, tag=f"maxq{k}", name=f"maxq{k}")
                for k in range(n_acc)
            ]
            for k in range(n_acc):
                nc.gpsimd.memset(maxq4[k][:], 0.0)

            # ---- per-GT chains ------------------------------------------
            for jj, (lo, hi) in enumerate(slices):
                if hi <= lo:
                    continue
                L = hi - lo
                S = slice(lo, hi)
                gx1, gy1, gx2, gy2, areag = gvals[jj]
                rs = rsp.tile([P, lmax], F32, tag="rs", name=f"rs{jj}")
                _scalar_act_raw(
                    nc, rs[:, :L], bbt[AREAB][:, S], ACT.Reciprocal, bias=areag
                )
                xd = work.tile([P, lmax], F32, tag="xd", name=f"xd{jj}")
                yd = work.tile([P, lmax], F32, tag="yd", name=f"yd{jj}")
                it = work.tile([P, lmax], F32, tag="it", name=f"it{jj}")
                qv = work.tile([P, lmax], F32, tag="qv", name=f"qv{jj}")
                nc.vector._custom_dve(
                    EXTENT, out=xd[:, :L], in0=bbt[BX2][:, S],
                    in1=bbt[BX1][:, S], s0=gx2, s1=gx1,
                )
                nc.vector._custom_dve(
                    EXTENT, out=yd[:, :L], in0=bbt[BY2][:, S],
                    in1=bbt[BY1][:, S], s0=gy2, s1=gy1,
                )
                nc.vector.tensor_tensor(
                    out=it[:, :L], in0=xd[:, :L], in1=yd[:, :L], op=AF.mult
                )
                nc.vector._custom_dve(
                    MUL_MAXRED, out=qv[:, :L], in0=it[:, :L], in1=rs[:, :L],
                    accum_out=colmax[:, jj : jj + 1],
                )
                mk = maxq4[jj % n_acc]
                nc.vector.tensor_tensor(
                    out=mk[:, S], in0=mk[:, S], in1=qv[:, :L], op=AF.max
                )

            # ---- fold accumulators, write outputs -----------------------
            maxw = constp.tile([P, cols], F32, tag="maxw", name="maxw")
            st = 1
            while st < n_acc:
                for a in range(0, n_acc, 2 * st):
                    dst = maxq4[a][:] if 2 * st < n_acc else maxw[:]
                    nc.vector.tensor_tensor(
                        out=dst, in0=maxq4[a][:], in1=maxq4[a + st][:], op=AF.max
                    )
                st *= 2
            nc.sync.dma_start(out_maxw[0:P], maxw[:])
            nc.sync.dma_start(out_small[0:P], colmax[:])

    nc.compile()
    return nc


# ----------------------------------------------------------------------------
# Host-side input prep / output gather / fixup
# ----------------------------------------------------------------------------
_NC_CACHE: dict = {}
LAST_RESULTS = None


def _iou_rows(bb_rows: np.ndarray, targets: np.ndarray, valid: np.ndarray):
    """Exact f32 replica of the reference IoU for a subset of anchors.

    bb_rows [F, 4], targets [G, 5] -> overlaps [F, G] f32 (invalid GTs -> -1).
    """
    f32 = np.float32
    fx1, fy1 = bb_rows[:, 0:1], bb_rows[:, 1:2]
    fx2, fy2 = bb_rows[:, 2:3], bb_rows[:, 3:4]
    tgx1, tgy1 = targets[None, :, 0], targets[None, :, 1]
    tgx2, tgy2 = targets[None, :, 2], targets[None, :, 3]
    iw = np.maximum(np.minimum(fx2, tgx2) - np.maximum(fx1, tgx1), f32(0)).astype(f32)
    ih = np.maximum(np.minimum(fy2, tgy2) - np.maximum(fy1, tgy1), f32(0)).astype(f32)
    fint = (iw * ih).astype(f32)
    fab = ((fx2 - fx1) * (fy2 - fy1)).astype(f32)
    fag = ((tgx2 - tgx1) * (tgy2 - tgy1)).astype(f32)
    fov = (fint / (fab + fag - fint + f32(1e-16))).astype(f32)
    return np.where(valid[None, :], fov, f32(-1.0))


def kernel(bboxes: np.ndarray, targets: np.ndarray, num_level_bboxes=None):
    f32 = np.float32
    bboxes = np.asarray(bboxes, dtype=f32)
    targets = np.asarray(targets, dtype=f32)
    n = bboxes.shape[0]
    assert n == N_FULL, f"kernel hardcoded for N={N_FULL}, got {n}"

    # Pad with degenerate far-away anchors (IoU 0 with every GT, y beyond
    # every slice).
    pad = np.full((N_PAD - n, 4), 2000.0, dtype=f32)
    bb_all = np.concatenate([bboxes, pad], axis=0)  # [N_PAD, 4]

    # y-sort anchors; rank r -> (col r//1024, core r%8, part (r%1024)//8)
    perm = np.argsort(bb_all[:, 1], kind="stable")
    bbs = bb_all[perm]
    ys = bbs[:, 1]
    maxhb = float((bboxes[:, 3] - bboxes[:, 1]).max()) + 1e-3

    # GT slot order: valid GTs sorted by gy1 (invalid get empty slices)
    lab = targets[:, 4]
    valid = lab != f32(-1.0)
    gy1key = np.where(valid, targets[:, 1], f32(1e9))
    gorder = np.argsort(gy1key, kind="stable")

    slices = []
    for j in gorder:
        if not valid[j]:
            slices.append((0, 0))
            continue
        gy1, gy2 = float(targets[j, 1]), float(targets[j, 3])
        lo = int(np.searchsorted(ys, gy1 - maxhb, "left")) // 1024
        hi = (int(np.searchsorted(ys, gy2, "right")) + 1023) // 1024
        hi = max(min(hi, C), 1)
        lo = max(0, min(lo, hi - 1))
        slices.append((lo, hi))
    slices = tuple(slices)

    # ---- device inputs ------------------------------------------------
    # bb [cores][5, P, C]: x1, y1, x2, y2, area_b
    arr = bbs.reshape(C, P, N_CORES, 4)  # [c, p, m, k]
    area_b = (
        (arr[..., 2] - arr[..., 0]) * (arr[..., 3] - arr[..., 1])
    ).astype(f32)  # [c, p, m]
    shards = []
    for m in range(N_CORES):
        sh = np.empty((5, P, C), dtype=f32)
        for k in range(4):
            sh[k] = arr[:, :, m, k].T
        sh[4] = area_b[:, :, m].T
        shards.append(sh)

    # GT scalars (slot = sorted order), baked into the program as imms.
    t = targets
    gx1 = t[gorder, 0].astype(f32)
    gy1 = t[gorder, 1].astype(f32)
    gx2 = t[gorder, 2].astype(f32)
    gy2 = t[gorder, 3].astype(f32)
    area_g = ((gx2 - gx1) * (gy2 - gy1)).astype(f32)
    gvals = tuple(
        (float(gx1[s]), float(gy1[s]), float(gx2[s]), float(gy2[s]), float(area_g[s]))
        for s in range(G)
    )

    key = (C, slices, gvals)
    if key not in _NC_CACHE:
        _NC_CACHE.clear()
        _NC_CACHE[key] = build_program(C, slices, gvals)
    nc = _NC_CACHE[key]
    in_maps = [{"bb": shards[m]} for m in range(N_CORES)]
    res = run_bass_kernel_spmd(nc, in_maps, core_ids=list(range(N_CORES)))
    global LAST_RESULTS
    LAST_RESULTS = res

    maxw_dev = np.stack([r["out_maxw"] for r in res.results])  # [m, P, C]
    small = np.stack([r["out_small"] for r in res.results])  # [m, P, G]

    # unshard maxw: sorted rank r = c*1024 + p*8 + m
    sorted_w = maxw_dev.transpose(2, 1, 0).reshape(N_PAD)
    w_full = np.empty_like(sorted_w)
    w_full[perm] = sorted_w
    w = w_full[:n].astype(np.float64)

    # w -> q conversion (w = q/(1+q)); device w has ~1.2e-5 rel error
    max_ov = (w / (1.0 - w)).astype(f32)

    # ---- host: thresholds with exact recompute near the boundaries ----
    flag = np.nonzero(
        (np.abs(max_ov - POS_THR) < THR_TOL) | (np.abs(max_ov - NEG_THR) < THR_TOL)
    )[0]
    if len(flag):
        fov = _iou_rows(bboxes[flag], targets, valid)
        max_ov[flag] = fov.max(1)

    pos_mask = max_ov > f32(POS_THR)
    neg_mask = max_ov < f32(NEG_THR)

    assigned = np.full(n, -1, dtype=np.int32)
    assigned[neg_mask] = 0

    # ---- host: exact argmax rows for the positive anchors -------------
    pos_idx = np.nonzero(pos_mask)[0]
    if len(pos_idx):
        fov = _iou_rows(bboxes[pos_idx], targets, valid)
        fmax = fov.max(1)
        farg = fov.argmax(1).astype(np.int32)
        max_ov[pos_idx] = fmax  # exact values for pos anchors
        # reference: pos if fmax > thr (exact); our w-approx agreed except
        # within THR_TOL which was already fixed exactly above
        assigned[pos_idx] = farg + 1

    # ---- host: the reference's per-GT overwrite pass -------------------
    # for j in 0..G-1 (valid, ascending): assigned[overlaps[:,j]==colmax_j]=j+1
    slot_of_j = np.empty(G, dtype=int)
    slot_of_j[gorder] = np.arange(G)
    arrv = bbs.reshape(C, P, N_CORES, 4)  # sorted-layout anchor coords
    for j in range(G):
        if not valid[j]:
            continue
        s = slot_of_j[j]
        col = small[:, :, s]  # [m, P] device w-space colmax
        glob = float(col.max())
        if glob <= 0.0:
            continue
        gx1j, gy1j, gx2j, gy2j = (float(targets[j, k]) for k in range(4))
        agj = np.float32(
            (np.float32(gx2j) - np.float32(gx1j))
            * (np.float32(gy2j) - np.float32(gy1j))
        )
        lo, hi = slices[s]
        for m, p in zip(*np.nonzero(col == glob)):
            row = arrv[lo:hi, p, m, :]  # [L, 4] f32
            iw = np.minimum(row[:, 2], np.float32(gx2j)) - np.maximum(
                row[:, 0], np.float32(gx1j)
            )
            ih = np.minimum(row[:, 3], np.float32(gy2j)) - np.maximum(
                row[:, 1], np.float32(gy1j)
            )
            iw = np.maximum(iw, np.float32(0)).astype(np.float32)
            ih = np.maximum(ih, np.float32(0)).astype(np.float32)
            inter_r = (iw * ih).astype(np.float32)
            ab = ((row[:, 2] - row[:, 0]) * (row[:, 3] - row[:, 1])).astype(
                np.float32
            )
            q = (inter_r / (ab + agj - inter_r)).astype(np.float32)
            c = lo + int(np.argmax(q))
            r = c * 1024 + int(p) * 8 + int(m)
            a = int(perm[r])
            if a < n:
                assigned[a] = j + 1

    labels = np.where(
        assigned > 0,
        lab[np.clip(assigned - 1, 0, G - 1)].astype(np.int32),
        -1,
    ).astype(np.int32)
    return assigned, max_ov, labels


if __name__ == "__main__":
    inp = {
        "bboxes": np.load("/root/problem/ref_bboxes.npy"),
        "targets": np.load("/root/problem/ref_targets.npy"),
        "num_level_bboxes": 5,
    }
    a, m, l = kernel(**inp)
    print("assigned", a[:10], "maxov", m[:5], "labels", l[:10])


# revision 9
# speedup vs baseline: 1.1476x; 1.0128x over previous
"""MaxIoUAssigner Trainium2 kernel (8 NeuronCores, SPMD over anchors).

Contract: kernel(**inputs) takes the FULL inputs
  bboxes  [500000, 4] f32
  targets [128, 5]    f32   (x1,y1,x2,y2,label; label==-1 => invalid GT)
  num_level_bboxes    (unused by the reference computation)
and returns the FULL outputs (assigned int32 [N], max_overlaps f32 [N],
assigned_labels int32 [N]) exactly like the jax reference.

Design v2 ("lean slab", DVE+ScalarE split):
  Anchors are y-sorted and laid out [128 partitions x C columns] per core
  (rank r -> col r//1024, core r%8, part (r%1024)//8). For each valid GT j
  (sorted by gy1) only a contiguous column slice [lo, hi) can overlap it.
  Per GT the device runs a 6-op chain over its slice in w-space
  (w = q/(1+q) = inter/(area_b+area_g), strictly monotone in IoU q):
    ScalarE: rs  = Reciprocal(area_b + area_g)   (fused act bias; ~1.2e-5 rel)
    DVE:     xd  = EXTENT(bx2,bx1 | gx2,gx1)     relu'd x-extent
             yd  = EXTENT(by2,by1 | gy2,gy1)
             it  = RELUMUL(xd, yd)               intersection
             q   = MUL_MAXRED(it, rs)            w values; accum -> colmax[P,1]
             acc = MAX2(acc, q)                  running row max (4 accums)
  The only outputs are maxw [P,C] (row max in w-space) and colmax [P,G].
  Everything else moved to the host: w->q conversion, pos/neg thresholds
  (threshold-window anchors recomputed exactly in f32), the row argmax for
  the ~2.7% positive anchors (exact f32 rows, reference tie semantics), and
  the reference's per-GT overwrite pass (device colmax selects the winning
  (core, partition); the winning column is recomputed exactly; top-2 colmax
  margins on this data are ~12x the device error).
"""

import sys

import numpy as np

sys.path.insert(0, "/opt/trn_rl_repo")

import concourse.bass as bass
import concourse.bacc as bacc
import concourse.mybir as mybir
from concourse import dve_ops
from concourse import tile
from concourse.bass_utils import run_bass_kernel_spmd
from concourse.dve_spec import Spec, Src0, Src1, Zero, lower, maxx, minn, relu
from concourse.dve_spec import C0 as DC0
from concourse.dve_spec import C1 as DC1
from concourse.dve_spec import _has_src1
from concourse.dve_uop import DveOpSpec
from concourse.dve_ops import DveOp

# ----------------------------------------------------------------------------
# Problem constants (hardcoded per the harness contract)
# ----------------------------------------------------------------------------
N_FULL = 500000
G = 128
N_CORES = 8
P = 128  # SBUF partitions
C = 489  # anchor columns per partition per core
N_CORE = P * C  # 62592 anchors per core (padded)
N_PAD = N_CORE * N_CORES  # 500736
POS_THR = 0.5
NEG_THR = 0.4
THR_TOL = 1e-4  # flag |q - thr| < tol for exact host recompute

F32 = mybir.dt.float32
AF = mybir.AluOpType
ACT = mybir.ActivationFunctionType


# ----------------------------------------------------------------------------
# Custom fused DVE ops (registered at import)
# ----------------------------------------------------------------------------
def _register_custom_op(name: str, spec: Spec, subdim: bool = False) -> DveOp:
    existing = {op.name: op for op in dve_ops.OPS}
    if name in existing:
        return existing[name]
    row = max(dve_ops._SUB_OPCODE_FOR_NAME.values()) + 1
    assert row < 0x20, "custom-DVE opcode rows exhausted"
    dve_ops._SUB_OPCODE_FOR_NAME[name] = row
    op = DveOp(name, spec, subdim=subdim, uops_sha={})
    for ver in ("v3", "v4"):
        tmp = DveOpSpec(
            name=name, opcode=row, uops=lower(spec, ver=ver), rd1_en=_has_src1(spec)
        )
        op.uops_sha[ver] = tmp.sha(ver)
    dve_ops.OPS.append(op)
    dve_ops.CUSTOM_DVE_SPECS[name] = spec
    return op


# clipped extent: relu(min(Src0, s0) - max(Src1, s1))
EXTENT = _register_custom_op(
    "IOU_EXTENT",
    Spec(
        body=relu(minn(Src0, DC0) - maxx(Src1, DC1)),
        reference=lambda in0, in1, c0, c1, c2: np.maximum(
            np.float32(np.minimum(in0, c0) - np.maximum(in1, c1)), np.float32(0)
        ),
    ),
)

# inter = relu(dx) * relu(dy)  (relu is a no-op here; extents already >=0)
RELUMUL = _register_custom_op(
    "IOU_RELUMUL",
    Spec(
        body=relu(Src0) * relu(Src1),
        reference=lambda in0, in1, c0, c1, c2: np.float32(
            np.maximum(in0, np.float32(0)) * np.maximum(in1, np.float32(0))
        ),
    ),
)

# elementwise max (row-max folding)
MAX2 = _register_custom_op(
    "IOU_MAX2",
    Spec(
        body=maxx(Src0, Src1),
        reference=lambda in0, in1, c0, c1, c2: np.maximum(in0, in1),
    ),
)

# out = Src0*Src1 ; accum_out = max(out) over the free dim (init 0)
MUL_MAXRED = _register_custom_op(
    "IOU_MUL_MAXRED",
    Spec(
        body=Src0 * Src1,
        accum=maxx,
        accum_init=Zero,
        reference=lambda in0, in1, c0, c1, c2: (
            r := np.float32(in0 * in1),
            np.max(r, axis=-1, keepdims=True),
        ),
    ),
)


def _scalar_act_raw(nc, out, in_, func, bias=0.0, scale=1.0, alpha=0.0):
    """Emit InstActivation directly (the bass wrapper forbids Reciprocal)."""
    eng = nc.scalar
    ins = [eng.lower_ap(in_)]
    for arg in (bias, scale, alpha):
        ins.append(mybir.ImmediateValue(dtype=mybir.dt.float32, value=float(arg)))
    return eng.add_instruction(
        mybir.InstActivation(
            name=nc.get_next_instruction_name(),
            func=func,
            ins=ins,
            outs=[eng.lower_ap(out)],
        )
    )


# ----------------------------------------------------------------------------
# Device program
# ----------------------------------------------------------------------------
def build_program(
    cols: int,
    slices: tuple,  # per sorted-GT (lo, hi); (0, 0) = invalid GT, skipped
    gvals: tuple,  # per sorted-GT (gx1, gy1, gx2, gy2, area_g) f32
) -> bass.Bass:
    """Per-core SPMD Bass program (identical on all cores; per-core data).

    bb  [5, P, cols]: x1, y1, x2, y2, area_b
    out_maxw  [P, cols]: row max in w-space
    out_small [P, G]:    per-GT core-local column max (w-space)
    """
    nc = bacc.Bacc(
        "TRN2", target_bir_lowering=False, debug=False, num_devices=N_CORES
    )

    bb = nc.declare_dram_parameter("bb", [5, P, cols], F32, isOutput=False)
    out_maxw = nc.declare_dram_parameter("out_maxw", [P, cols], F32, isOutput=True)
    out_small = nc.declare_dram_parameter("out_small", [P, G], F32, isOutput=True)

    BX1, BY1, BX2, BY2, AREAB = range(5)

    lmax = max([hi - lo for (lo, hi) in slices] + [1])
    n_acc = 4  # independent running-max accumulators

    with tile.TileContext(nc) as tc:
        with (
            tc.tile_pool(name="const", bufs=1) as constp,
            tc.tile_pool(name="rsp", bufs=8) as rsp,
            tc.tile_pool(name="work", bufs=6) as work,
        ):
            # ---- constants / inputs -------------------------------------
            # chunked plane DMAs, extent planes first, so the first GT
            # chains start as soon as their columns have landed
            bbt = [
                constp.tile([P, cols], F32, tag=f"bb{k}", name=f"bb{k}")
                for k in range(5)
            ]
            half = cols // 2
            for k in (2, 0, 3, 1, 4):  # x2, x1, y2, y1, area_b
                nc.sync.dma_start(bbt[k][:, :half], bb[k][:, :half])
            for k in (2, 0, 3, 1, 4):
                nc.sync.dma_start(bbt[k][:, half:], bb[k][:, half:])

            colmax = constp.tile([P, G], F32, tag="colmax", name="colmax")
            nc.scalar.memzero(colmax[:])
            maxq4 = [
                constp.tile([P, cols], F32, tag=f"maxq{k}", name=f"maxq{k}")
                for k in range(n_acc)
            ]
            for k in range(n_acc):
                nc.scalar.memzero(maxq4[k][:])

            # ---- per-GT chains ------------------------------------------
            for jj, (lo, hi) in enumerate(slices):
                if hi <= lo:
                    continue
                L = hi - lo
                S = slice(lo, hi)
                gx1, gy1, gx2, gy2, areag = gvals[jj]
                rs = rsp.tile([P, lmax], F32, tag="rs", name=f"rs{jj}")
                _scalar_act_raw(
                    nc, rs[:, :L], bbt[AREAB][:, S], ACT.Reciprocal, bias=areag
                )
                xd = work.tile([P, lmax], F32, tag="xd", name=f"xd{jj}")
                yd = work.tile([P, lmax], F32, tag="yd", name=f"yd{jj}")
                it = work.tile([P, lmax], F32, tag="it", name=f"it{jj}")
                qv = work.tile([P, lmax], F32, tag="qv", name=f"qv{jj}")
                nc.vector._custom_dve(
                    EXTENT, out=xd[:, :L], in0=bbt[BX2][:, S],
                    in1=bbt[BX1][:, S], s0=gx2, s1=gx1,
                )
                nc.vector._custom_dve(
                    EXTENT, out=yd[:, :L], in0=bbt[BY2][:, S],
                    in1=bbt[BY1][:, S], s0=gy2, s1=gy1,
                )
                nc.vector.tensor_tensor(
                    out=it[:, :L], in0=xd[:, :L], in1=yd[:, :L], op=AF.mult
                )
                nc.vector._custom_dve(
                    MUL_MAXRED, out=qv[:, :L], in0=it[:, :L], in1=rs[:, :L],
                    accum_out=colmax[:, jj : jj + 1],
                )
                mk = maxq4[jj % n_acc]
                nc.vector.tensor_tensor(
                    out=mk[:, S], in0=mk[:, S], in1=qv[:, :L], op=AF.max
                )

            # ---- fold accumulators, write outputs -----------------------
            maxw = constp.tile([P, cols], F32, tag="maxw", name="maxw")
            st = 1
            while st < n_acc:
                for a in range(0, n_acc, 2 * st):
                    dst = maxq4[a][:] if 2 * st < n_acc else maxw[:]
                    nc.vector.tensor_tensor(
                        out=dst, in0=maxq4[a][:], in1=maxq4[a + st][:], op=AF.max
                    )
                st *= 2
            nc.sync.dma_start(out_maxw[0:P], maxw[:])
            nc.sync.dma_start(out_small[0:P], colmax[:])

    nc.compile()
    return nc


# ----------------------------------------------------------------------------
# Host-side input prep / output gather / fixup
# ----------------------------------------------------------------------------
_NC_CACHE: dict = {}
LAST_RESULTS = None


def _iou_rows(bb_rows: np.ndarray, targets: np.ndarray, valid: np.ndarray):
    """Exact f32 replica of the reference IoU for a subset of anchors.

    bb_rows [F, 4], targets [G, 5] -> overlaps [F, G] f32 (invalid GTs -> -1).
    """
    f32 = np.float32
    fx1, fy1 = bb_rows[:, 0:1], bb_rows[:, 1:2]
    fx2, fy2 = bb_rows[:, 2:3], bb_rows[:, 3:4]
    tgx1, tgy1 = targets[None, :, 0], targets[None, :, 1]
    tgx2, tgy2 = targets[None, :, 2], targets[None, :, 3]
    iw = np.maximum(np.minimum(fx2, tgx2) - np.maximum(fx1, tgx1), f32(0)).astype(f32)
    ih = np.maximum(np.minimum(fy2, tgy2) - np.maximum(fy1, tgy1), f32(0)).astype(f32)
    fint = (iw * ih).astype(f32)
    fab = ((fx2 - fx1) * (fy2 - fy1)).astype(f32)
    fag = ((tgx2 - tgx1) * (tgy2 - tgy1)).astype(f32)
    fov = (fint / (fab + fag - fint + f32(1e-16))).astype(f32)
    return np.where(valid[None, :], fov, f32(-1.0))


def kernel(bboxes: np.ndarray, targets: np.ndarray, num_level_bboxes=None):
    f32 = np.float32
    bboxes = np.asarray(bboxes, dtype=f32)
    targets = np.asarray(targets, dtype=f32)
    n = bboxes.shape[0]
    assert n == N_FULL, f"kernel hardcoded for N={N_FULL}, got {n}"

    # Pad with degenerate far-away anchors (IoU 0 with every GT, y beyond
    # every slice).
    pad = np.full((N_PAD - n, 4), 2000.0, dtype=f32)
    bb_all = np.concatenate([bboxes, pad], axis=0)  # [N_PAD, 4]

    # y-sort anchors; rank r -> (col r//1024, core r%8, part (r%1024)//8)
    perm = np.argsort(bb_all[:, 1], kind="stable")
    bbs = bb_all[perm]
    ys = bbs[:, 1]
    maxhb = float((bboxes[:, 3] - bboxes[:, 1]).max()) + 1e-3

    # GT slot order: valid GTs sorted by gy1 (invalid get empty slices)
    lab = targets[:, 4]
    valid = lab != f32(-1.0)
    gy1key = np.where(valid, targets[:, 1], f32(1e9))
    gorder = np.argsort(gy1key, kind="stable")

    slices = []
    for j in gorder:
        if not valid[j]:
            slices.append((0, 0))
            continue
        gy1, gy2 = float(targets[j, 1]), float(targets[j, 3])
        lo = int(np.searchsorted(ys, gy1 - maxhb, "left")) // 1024
        hi = (int(np.searchsorted(ys, gy2, "right")) + 1023) // 1024
        hi = max(min(hi, C), 1)
        lo = max(0, min(lo, hi - 1))
        slices.append((lo, hi))
    slices = tuple(slices)

    # ---- device inputs ------------------------------------------------
    # bb [cores][5, P, C]: x1, y1, x2, y2, area_b
    arr = bbs.reshape(C, P, N_CORES, 4)  # [c, p, m, k]
    area_b = (
        (arr[..., 2] - arr[..., 0]) * (arr[..., 3] - arr[..., 1])
    ).astype(f32)  # [c, p, m]
    shards = []
    for m in range(N_CORES):
        sh = np.empty((5, P, C), dtype=f32)
        for k in range(4):
            sh[k] = arr[:, :, m, k].T
        sh[4] = area_b[:, :, m].T
        shards.append(sh)

    # GT scalars (slot = sorted order), baked into the program as imms.
    t = targets
    gx1 = t[gorder, 0].astype(f32)
    gy1 = t[gorder, 1].astype(f32)
    gx2 = t[gorder, 2].astype(f32)
    gy2 = t[gorder, 3].astype(f32)
    area_g = ((gx2 - gx1) * (gy2 - gy1)).astype(f32)
    gvals = tuple(
        (float(gx1[s]), float(gy1[s]), float(gx2[s]), float(gy2[s]), float(area_g[s]))
        for s in range(G)
    )

    key = (C, slices, gvals)
    if key not in _NC_CACHE:
        _NC_CACHE.clear()
        _NC_CACHE[key] = build_program(C, slices, gvals)
    nc = _NC_CACHE[key]
    in_maps = [{"bb": shards[m]} for m in range(N_CORES)]
    res = run_bass_kernel_spmd(nc, in_maps, core_ids=list(range(N_CORES)))
    global LAST_RESULTS
    LAST_RESULTS = res

    maxw_dev = np.stack([r["out_maxw"] for r in res.results])  # [m, P, C]
    small = np.stack([r["out_small"] for r in res.results])  # [m, P, G]

    # unshard maxw: sorted rank r = c*1024 + p*8 + m
    sorted_w = maxw_dev.transpose(2, 1, 0).reshape(N_PAD)
    w_full = np.empty_like(sorted_w)
    w_full[perm] = sorted_w
    w = w_full[:n].astype(np.float64)

    # w -> q conversion (w = q/(1+q)); device w has ~1.2e-5 rel error
    max_ov = (w / (1.0 - w)).astype(f32)

    # ---- host: thresholds with exact recompute near the boundaries ----
    flag = np.nonzero(
        (np.abs(max_ov - POS_THR) < THR_TOL) | (np.abs(max_ov - NEG_THR) < THR_TOL)
    )[0]
    if len(flag):
        fov = _iou_rows(bboxes[flag], targets, valid)
        max_ov[flag] = fov.max(1)

    pos_mask = max_ov > f32(POS_THR)
    neg_mask = max_ov < f32(NEG_THR)

    assigned = np.full(n, -1, dtype=np.int32)
    assigned[neg_mask] = 0

    # ---- host: exact argmax rows for the positive anchors -------------
    pos_idx = np.nonzero(pos_mask)[0]
    if len(pos_idx):
        fov = _iou_rows(bboxes[pos_idx], targets, valid)
        fmax = fov.max(1)
        farg = fov.argmax(1).astype(np.int32)
        max_ov[pos_idx] = fmax  # exact values for pos anchors
        # reference: pos if fmax > thr (exact); our w-approx agreed except
        # within THR_TOL which was already fixed exactly above
        assigned[pos_idx] = farg + 1

    # ---- host: the reference's per-GT overwrite pass -------------------
    # for j in 0..G-1 (valid, ascending): assigned[overlaps[:,j]==colmax_j]=j+1
    slot_of_j = np.empty(G, dtype=int)
    slot_of_j[gorder] = np.arange(G)
    arrv = bbs.reshape(C, P, N_CORES, 4)  # sorted-layout anchor coords
    for j in range(G):
        if not valid[j]:
            continue
        s = slot_of_j[j]
        col = small[:, :, s]  # [m, P] device w-space colmax
        glob = float(col.max())
        if glob <= 0.0:
            continue
        gx1j, gy1j, gx2j, gy2j = (float(targets[j, k]) for k in range(4))
        agj = np.float32(
            (np.float32(gx2j) - np.float32(gx1j))
            * (np.float32(gy2j) - np.float32(gy1j))
        )
        lo, hi = slices[s]
        for m, p in zip(*np.nonzero(col == glob)):
            row = arrv[lo:hi, p, m, :]  # [L, 4] f32
            iw = np.minimum(row[:, 2], np.float32(gx2j)) - np.maximum(
                row[:, 0], np.float32(gx1j)
            )
            ih = np.minimum(row[:, 3], np.float32(gy2j)) - np.maximum(
                row[:, 1], np.float32(gy1j)
            )
            iw = np.maximum(iw, np.float32(0)).astype(np.float32)
            ih = np.maximum(ih, np.float32(0)).astype(np.float32)
            inter_r = (iw * ih).astype(np.float32)
            ab = ((row[:, 2] - row[:, 0]) * (row[:, 3] - row[:, 1])).astype(
                np.float32
            )
            q = (inter_r / (ab + agj - inter_r)).astype(np.float32)
            c = lo + int(np.argmax(q))
            r = c * 1024 + int(p) * 8 + int(m)
            a = int(perm[r])
            if a < n:
                assigned[a] = j + 1

    labels = np.where(
        assigned > 0,
        lab[np.clip(assigned - 1, 0, G - 1)].astype(np.int32),
        -1,
    ).astype(np.int32)
    return assigned, max_ov, labels


if __name__ == "__main__":
    inp = {
        "bboxes": np.load("/root/problem/ref_bboxes.npy"),
        "targets": np.load("/root/problem/ref_targets.npy"),
        "num_level_bboxes": 5,
    }
    a, m, l = kernel(**inp)
    print("assigned", a[:10], "maxov", m[:5], "labels", l[:10])
